# revision 2
# baseline (speedup 1.0000x reference)
"""Trainium2 Bass kernel for nn_EstraNet_1443109012284.

Mathematical reduction: the reference's FAVOR+/trig branch (phi_q, aux_q/k,
fr_q/k, aux_A, A) does not feed the output.  The output is exactly

    out[b,n,d] = sum_{h,c} W_o[h,c,d] * norma[h] * sum_{d'} W_v[d',h,c] * x[b,n,d']
               = (x @ M)[b,n,d],   M[d',d] = sum_{h,c} W_v[d',h,c] norma[h] W_o[h,c,d]

with norma[h] = || sum_d s_p[h] W_p[d,h,:] beta_p[d] ||_2.

M is a tiny [512,512] matrix folded on the host; the device does the single
big GEMM  y[32768,512] = x[32768,512] @ M[512,512]  data-parallel over rows:
each of the 8 cores handles 4096 rows (yT[d, n] = sum_k M[k, d] xT[k, n]).

v2 schedule (per core), built from the v1 profile:
- 8 column-stripes of 512; per stripe 16 MMs (4 k-chunks x 4 d-blocks),
  k-outer so each of the 4 PSUM banks accumulates across the whole stripe
  and input-chunk deadlines are maximally late.  Two stripes in flight
  use all 8 PSUM banks; a stripe's banks drain (ACT copy -> fp16 tile ->
  DMA) while the next stripe computes.  Last stripe runs d-outer/k-inner
  so its banks retire staggered and the tail is short.
- The PE p-state ramp (~3.6us at half clock from the first matmul) is
  burned with a few warmup MMs on a memset tile, switching to real MMs
  as soon as the first m/x chunks land: real work proceeds at half rate
  during the remainder of the ramp instead of idling behind warmups.
- M is split into 4 k-chunks issued FIRST on each DMA queue (SP HW,
  ACT HW, Pool SW) so the first real matmul only waits for one 128KB
  chunk, not the whole 512KB of M.
- Input x is spread over all 3 queues in consumption order; first two
  stripes as 128KB chunks (early availability), rest as 256KB chunks
  (issue-cost amortization).  Outputs go to the SP HW queue (stripe 4 to
  the Pool SW queue to cover SP's input backlog) so the final stripes
  drain on an empty queue.
- fp16 end-to-end (x, M, y), M pre-scaled by a power of two on the host
  so values clear the fp16 subnormal range; the scale is divided back out
  on the host.
"""

import os as _os
import sys

sys.path.insert(0, "/opt/trn_rl_repo")

import numpy as np

import concourse.bass as bass
import concourse.tile as tile
from concourse import bacc, mybir
from concourse.bass_utils import run_bass_kernel_spmd

N_CORES = 8
ROWS = 32768           # B*N = 8*4096
RPC = ROWS // N_CORES  # rows per core = 4096
D = 512
KC = 4                 # contraction chunks of 128
DT = D // 128          # output row-blocks = 4
NS = 8                 # column stripes per core
SW = RPC // NS         # stripe width = 512

COMPUTE_DTYPE = _os.environ.get("KERNEL_DTYPE", "fp16")
N_WARM = int(_os.environ.get("KERNEL_NWARM", "5"))

_DT = {
    "fp32": mybir.dt.float32,
    "f32r": mybir.dt.float32r,
    "bf16": mybir.dt.bfloat16,
    "fp16": mybir.dt.float16,
}


def _np_dtype(token):
    if token == "bf16":
        import ml_dtypes

        return ml_dtypes.bfloat16
    if token == "fp16":
        return np.float16
    return np.float32


def _build(token):
    dt_in = _DT[token]
    dt_out = mybir.dt.float16 if token == "fp16" else mybir.dt.float32
    nc = bacc.Bacc("TRN2", target_bir_lowering=False)
    # x pre-transposed on the host.  Stripes 0-1 as [k, stripe] 128KB
    # chunks; stripes 2-7 as [k, stripe-pair] 256KB chunks.
    xs0 = nc.dram_tensor("xs0", [KC, 2, 128, SW], dt_in, kind="ExternalInput")
    xbig = nc.dram_tensor("xbig", [KC, 3, 128, 2 * SW], dt_in, kind="ExternalInput")
    mm = nc.dram_tensor("mm", [KC, 128, D], dt_in, kind="ExternalInput")
    yt = nc.dram_tensor("yt", [D, RPC], dt_out, kind="ExternalOutput")

    with tile.TileContext(nc) as tc:
        with (
            tc.tile_pool(name="wp", bufs=1) as wp,
            tc.tile_pool(name="mp", bufs=1) as mp,
            tc.tile_pool(name="xp", bufs=1) as xp,
            tc.tile_pool(name="op", bufs=8) as op,
            tc.tile_pool(name="pp", bufs=8, space="PSUM") as pp,
        ):
            # Warmup MMs start the PE p-state ramp immediately; they only
            # depend on a DVE memset (DVE is otherwise idle; gpsimd must
            # start issuing SW-queue DMAs right away).
            wz = wp.tile([128, 512], mybir.dt.bfloat16, name="wz")
            nc.vector.memset(wz[:], 1.0)
            warm = pp.tile([128, 512], mybir.dt.float32, tag="ps", name="warm")
            for _ in range(N_WARM):
                nc.tensor.matmul(warm[:], wz[:, 0:128], wz[:], start=True, stop=True)

            # --- input DMA issue streams (order per engine == issue order) ---
            m_sb = [mp.tile([128, D], dt_in, tag=f"m{k}", name=f"m{k}") for k in range(KC)]
            # stripe 0-1 x tiles: [128, 512] per (k, s)
            xt_s01 = {}
            for k in range(KC):
                for s in range(2):
                    xt_s01[(k, s)] = xp.tile([128, SW], dt_in, tag=f"a{k}{s}", name=f"a{k}{s}")
            # stripe-pair tiles for ss = 1, 2, 3 (stripes 2..7)
            xt_big = {}
            for k in range(KC):
                for ss in range(1, 4):
                    xt_big[(k, ss)] = xp.tile([128, 2 * SW], dt_in, tag=f"b{k}{ss}", name=f"b{k}{ss}")

            # SP HW queue: m0, m1, then k-odd chunks in consumption order
            nc.sync.dma_start(out=m_sb[0][:], in_=mm[0])
            nc.sync.dma_start(out=m_sb[1][:], in_=mm[1])
            nc.sync.dma_start(out=xt_s01[(1, 0)][:], in_=xs0[1, 0])
            nc.sync.dma_start(out=xt_s01[(3, 0)][:], in_=xs0[3, 0])
            nc.sync.dma_start(out=xt_s01[(1, 1)][:], in_=xs0[1, 1])
            nc.sync.dma_start(out=xt_s01[(3, 1)][:], in_=xs0[3, 1])
            nc.sync.dma_start(out=xt_big[(1, 1)][:], in_=xbig[1, 0])
            nc.sync.dma_start(out=xt_big[(3, 1)][:], in_=xbig[3, 0])
            # ACT HW queue: k-even chunks of stripes 0-3
            nc.scalar.dma_start(out=xt_s01[(0, 0)][:], in_=xs0[0, 0])
            nc.scalar.dma_start(out=xt_s01[(2, 0)][:], in_=xs0[2, 0])
            nc.scalar.dma_start(out=xt_s01[(0, 1)][:], in_=xs0[0, 1])
            nc.scalar.dma_start(out=xt_s01[(2, 1)][:], in_=xs0[2, 1])
            nc.scalar.dma_start(out=xt_big[(0, 1)][:], in_=xbig[0, 0])
            nc.scalar.dma_start(out=xt_big[(2, 1)][:], in_=xbig[2, 0])
            # Pool SW queue: m2, m3, then all of stripes 4-7
            nc.gpsimd.dma_start(out=m_sb[2][:], in_=mm[2])
            nc.gpsimd.dma_start(out=m_sb[3][:], in_=mm[3])
            for ss in (2, 3):
                for k in range(KC):
                    nc.gpsimd.dma_start(out=xt_big[(k, ss)][:], in_=xbig[k, ss - 1])

            def xslice(k, s):
                if s < 2:
                    return xt_s01[(k, s)][:]
                ss, par = divmod(s, 2)
                return xt_big[(k, ss)][:, par * SW : (par + 1) * SW]

            # --- compute + drain ---
            for s in range(NS):
                pss = [
                    pp.tile([128, SW], mybir.dt.float32, tag="ps", name=f"ps_{s}_{d}")
                    for d in range(DT)
                ]
                if s < NS - 1:
                    # k-outer: latest possible input deadlines, banks
                    # complete in the stripe's last 4 MMs
                    for k in range(KC):
                        for d in range(DT):
                            nc.tensor.matmul(
                                pss[d][:],
                                m_sb[k][:, d * 128 : (d + 1) * 128],
                                xslice(k, s),
                                start=(k == 0),
                                stop=(k == KC - 1),
                            )
                else:
                    # last stripe d-outer: banks retire staggered -> short tail
                    for d in range(DT):
                        for k in range(KC):
                            nc.tensor.matmul(
                                pss[d][:],
                                m_sb[k][:, d * 128 : (d + 1) * 128],
                                xslice(k, s),
                                start=(k == 0),
                                stop=(k == KC - 1),
                            )
                oeng = nc.gpsimd if s == 4 else nc.sync
                for d in range(DT):
                    ot = op.tile([128, SW], dt_out, tag="ot", name=f"ot{s}_{d}")
                    nc.scalar.copy(ot[:], pss[d][:])
                    oeng.dma_start(
                        out=yt[d * 128 : (d + 1) * 128, s * SW : (s + 1) * SW],
                        in_=ot[:],
                    )
    nc.compile()
    return nc


def _fold_m(W_v, s_p, W_p, beta_p, W_o):
    """Host-side constant folding of the tiny parameter tensors into M."""
    W_v = np.asarray(W_v, dtype=np.float64)
    s_p = np.asarray(s_p, dtype=np.float64)
    W_p = np.asarray(W_p, dtype=np.float64)
    beta_p = np.asarray(beta_p, dtype=np.float64)
    W_o = np.asarray(W_o, dtype=np.float64)
    phi = np.einsum("h,dhc,d->hc", s_p, W_p, beta_p)
    norma = np.linalg.norm(phi, axis=1)  # [h]
    M = np.einsum("dhc,h,hce->de", W_v, norma, W_o)  # [512, 512]
    return M.astype(np.float32)


_prog_cache = {}
_last_in_maps = None  # kept for test.py profiling reuse
_last_result = None


def _run(in_maps, token, **kwargs):
    if token not in _prog_cache:
        _prog_cache[token] = _build(token)
    return run_bass_kernel_spmd(_prog_cache[token], in_maps, list(range(N_CORES)), **kwargs)


def kernel(x, W_v, s_p, c_p, W_p, W_A, W_o, beta_p, beta_i_p, **_unused):
    global _last_in_maps, _last_result
    token = COMPUTE_DTYPE
    np_dt = _np_dtype(token)

    x = np.asarray(x, dtype=np.float32)
    M = _fold_m(W_v, s_p, W_p, beta_p, W_o)

    # fp16 path: scale M by an exact power of two so M entries and y values
    # sit in fp16 normal range; undo on the host after the run
    out_unscale = 1.0
    if token == "fp16":
        amax = float(np.abs(M).max())
        if amax > 0:
            e = int(np.floor(-np.log2(amax)))
            M = M * np.float32(2.0**e)
            out_unscale = 2.0**-e

    B, N, Dd = x.shape
    assert B * N == ROWS and Dd == D, (x.shape,)

    mmc = np.ascontiguousarray(M.reshape(KC, 128, D)).astype(np_dt)
    xf = x.reshape(ROWS, D)

    in_maps = []
    for c in range(N_CORES):
        sh = xf[c * RPC : (c + 1) * RPC]               # [4096, 512]
        xT = sh.T.astype(np_dt)                        # [512, 4096]
        xk = xT.reshape(KC, 128, NS, SW)               # [k, part, stripe, col]
        xs0 = np.ascontiguousarray(xk[:, :, 0:2].transpose(0, 2, 1, 3))
        xbig = np.ascontiguousarray(
            xk[:, :, 2:].reshape(KC, 128, 3, 2 * SW).transpose(0, 2, 1, 3)
        )
        in_maps.append({"xs0": xs0, "xbig": xbig, "mm": mmc})

    _last_in_maps = in_maps
    res = _run(in_maps, token)
    _last_result = res
    out = np.empty((ROWS, D), dtype=np.float32)
    for c in range(N_CORES):
        yc = res.results[c]["yt"].astype(np.float32)
        if out_unscale != 1.0:
            yc *= np.float32(out_unscale)
        out[c * RPC : (c + 1) * RPC] = yc.T
    return out.reshape(B, N, D)


if __name__ == "__main__":
    # smoke test with random data
    rng = np.random.default_rng(0)
    x = rng.standard_normal((8, 4096, 512)).astype(np.float32)
    W_v = rng.standard_normal((512, 8, 64)).astype(np.float32) * 0.01
    s_p = np.ones((8,), np.float32)
    c_p = np.ones((8,), np.float32)
    W_p = rng.standard_normal((512, 8, 64)).astype(np.float32) * 0.01
    W_A = rng.standard_normal((256, 64)).astype(np.float32)
    W_o = rng.standard_normal((8, 64, 512)).astype(np.float32) * 0.01
    beta_p = rng.standard_normal((512,)).astype(np.float32) * 1e-5
    beta_i_p = rng.standard_normal((4096, 512)).astype(np.float32) * 1e-5
    out = kernel(x, W_v=W_v, s_p=s_p, c_p=c_p, W_p=W_p, W_A=W_A, W_o=W_o,
                 beta_p=beta_p, beta_i_p=beta_i_p)
    M = _fold_m(W_v, s_p, W_p, beta_p, W_o)
    exp = (x.reshape(-1, 512).astype(np.float64) @ M.astype(np.float64)).reshape(8, 4096, 512)
    err = np.abs(out - exp).max() / (np.abs(exp).max() + 1e-30)
    print("smoke rel err:", err)


# revision 5
# speedup vs baseline: 1.0086x; 1.0086x over previous
"""Trainium2 Bass kernel for nn_EstraNet_1443109012284.

Mathematical reduction: the reference's FAVOR+/trig branch (phi_q, aux_q/k,
fr_q/k, aux_A, A) does not feed the output.  The output is exactly

    out[b,n,d] = sum_{h,c} W_o[h,c,d] * norma[h] * sum_{d'} W_v[d',h,c] * x[b,n,d']
               = (x @ M)[b,n,d],   M[d',d] = sum_{h,c} W_v[d',h,c] norma[h] W_o[h,c,d]

with norma[h] = || sum_d s_p[h] W_p[d,h,:] beta_p[d] ||_2.

M is a tiny [512,512] matrix folded on the host; the device does the single
big GEMM  y[32768,512] = x[32768,512] @ M[512,512]  data-parallel over rows:
each of the 8 cores handles 4096 rows (yT[d, n] = sum_k M[k, d] xT[k, n]).

v2 schedule (per core), built from the v1 profile:
- 8 column-stripes of 512; per stripe 16 MMs (4 k-chunks x 4 d-blocks),
  k-outer so each of the 4 PSUM banks accumulates across the whole stripe
  and input-chunk deadlines are maximally late.  Two stripes in flight
  use all 8 PSUM banks; a stripe's banks drain (ACT copy -> fp16 tile ->
  DMA) while the next stripe computes.  Last stripe runs d-outer/k-inner
  so its banks retire staggered and the tail is short.
- The PE p-state ramp (~3.6us at half clock from the first matmul) is
  burned with a few warmup MMs on a memset tile, switching to real MMs
  as soon as the first m/x chunks land: real work proceeds at half rate
  during the remainder of the ramp instead of idling behind warmups.
- M is split into 4 k-chunks issued FIRST on each DMA queue (SP HW,
  ACT HW, Pool SW) so the first real matmul only waits for one 128KB
  chunk, not the whole 512KB of M.
- Input x is spread over all 3 queues in consumption order; first two
  stripes as 128KB chunks (early availability), rest as 256KB chunks
  (issue-cost amortization).  Outputs go to the SP HW queue (stripe 4 to
  the Pool SW queue to cover SP's input backlog) so the final stripes
  drain on an empty queue.
- fp16 end-to-end (x, M, y), M pre-scaled by a power of two on the host
  so values clear the fp16 subnormal range; the scale is divided back out
  on the host.
"""

import os as _os
import sys

sys.path.insert(0, "/opt/trn_rl_repo")

import numpy as np

import concourse.bass as bass
import concourse.tile as tile
from concourse import bacc, mybir
from concourse.bass_utils import run_bass_kernel_spmd

N_CORES = 8
ROWS = 32768           # B*N = 8*4096
RPC = ROWS // N_CORES  # rows per core = 4096
D = 512
KC = 4                 # contraction chunks of 128
DT = D // 128          # output row-blocks = 4
NS = 8                 # column stripes per core
SW = RPC // NS         # stripe width = 512

COMPUTE_DTYPE = _os.environ.get("KERNEL_DTYPE", "fp16")
N_WARM = int(_os.environ.get("KERNEL_NWARM", "5"))

_DT = {
    "fp32": mybir.dt.float32,
    "f32r": mybir.dt.float32r,
    "bf16": mybir.dt.bfloat16,
    "fp16": mybir.dt.float16,
}


def _np_dtype(token):
    if token == "bf16":
        import ml_dtypes

        return ml_dtypes.bfloat16
    if token == "fp16":
        return np.float16
    return np.float32


def _build(token):
    dt_in = _DT[token]
    dt_out = mybir.dt.float16 if token == "fp16" else mybir.dt.float32
    nc = bacc.Bacc("TRN2", target_bir_lowering=False)
    # x pre-transposed on the host.  Stripes 0-1 as [k, stripe] 128KB
    # chunks (early availability); stripes 2-4 / 5-7 as [k, 1536] 384KB
    # chunks (amortize the ~0.65us HWDGE / ~1.0us SWDGE issue cost).
    xs0 = nc.dram_tensor("xs0", [KC, 2, 128, SW], dt_in, kind="ExternalInput")
    xmid = nc.dram_tensor("xmid", [KC, 128, 3 * SW], dt_in, kind="ExternalInput")
    xend = nc.dram_tensor("xend", [KC, 128, 3 * SW], dt_in, kind="ExternalInput")
    mm = nc.dram_tensor("mm", [KC, 128, D], dt_in, kind="ExternalInput")
    yt = nc.dram_tensor("yt", [D, RPC], dt_out, kind="ExternalOutput")

    with tile.TileContext(nc) as tc:
        with (
            tc.tile_pool(name="wp", bufs=1) as wp,
            tc.tile_pool(name="mp", bufs=1) as mp,
            tc.tile_pool(name="xp", bufs=1) as xp,
            tc.tile_pool(name="op", bufs=1) as op,
            tc.tile_pool(name="pp", bufs=8, space="PSUM") as pp,
        ):
            # Warmup MMs start the PE p-state ramp immediately; they only
            # depend on a DVE memset (DVE is otherwise idle; gpsimd must
            # start issuing SW-queue DMAs right away).
            wz = wp.tile([128, 512], mybir.dt.bfloat16, name="wz")
            nc.vector.memset(wz[:], 1.0)
            warm = pp.tile([128, 512], mybir.dt.float32, tag="ps", name="warm")
            for _ in range(N_WARM):
                nc.tensor.matmul(warm[:], wz[:, 0:128], wz[:], start=True, stop=True)

            # --- input tiles ---
            m_sb = [mp.tile([128, D], dt_in, tag=f"m{k}", name=f"m{k}") for k in range(KC)]
            xt_s01 = {}
            for k in range(KC):
                for s in range(2):
                    xt_s01[(k, s)] = xp.tile([128, SW], dt_in, tag=f"a{k}{s}", name=f"a{k}{s}")
            xt_mid = {}
            xt_end = {}
            for k in range(KC):
                xt_mid[k] = xp.tile([128, 3 * SW], dt_in, tag=f"mid{k}", name=f"mid{k}")
                xt_end[k] = xp.tile([128, 3 * SW], dt_in, tag=f"end{k}", name=f"end{k}")

            # --- input DMA issue streams (order per engine == issue order) ---
            # SP HW queue
            nc.sync.dma_start(out=m_sb[0][:], in_=mm[0])
            nc.sync.dma_start(out=m_sb[1][:], in_=mm[1])
            nc.sync.dma_start(out=xt_s01[(1, 0)][:], in_=xs0[1, 0])
            nc.sync.dma_start(out=xt_s01[(3, 0)][:], in_=xs0[3, 0])
            nc.sync.dma_start(out=xt_s01[(1, 1)][:], in_=xs0[1, 1])
            nc.sync.dma_start(out=xt_s01[(3, 1)][:], in_=xs0[3, 1])
            nc.sync.dma_start(out=xt_mid[2][:], in_=xmid[2])
            nc.sync.dma_start(out=xt_end[1][:], in_=xend[1])
            nc.sync.dma_start(out=xt_end[3][:], in_=xend[3])
            # ACT HW queue: only the 4 earliest chunks; ACT then copies
            nc.scalar.dma_start(out=xt_s01[(0, 0)][:], in_=xs0[0, 0])
            nc.scalar.dma_start(out=xt_s01[(2, 0)][:], in_=xs0[2, 0])
            nc.scalar.dma_start(out=xt_s01[(0, 1)][:], in_=xs0[0, 1])
            nc.scalar.dma_start(out=xt_s01[(2, 1)][:], in_=xs0[2, 1])
            # Pool SW queue
            nc.gpsimd.dma_start(out=m_sb[2][:], in_=mm[2])
            nc.gpsimd.dma_start(out=m_sb[3][:], in_=mm[3])
            nc.gpsimd.dma_start(out=xt_mid[0][:], in_=xmid[0])
            nc.gpsimd.dma_start(out=xt_mid[1][:], in_=xmid[1])
            nc.gpsimd.dma_start(out=xt_mid[3][:], in_=xmid[3])
            nc.gpsimd.dma_start(out=xt_end[0][:], in_=xend[0])
            nc.gpsimd.dma_start(out=xt_end[2][:], in_=xend[2])

            def xslice(k, s):
                if s < 2:
                    return xt_s01[(k, s)][:]
                if s < 5:
                    return xt_mid[k][:, (s - 2) * SW : (s - 1) * SW]
                return xt_end[k][:, (s - 5) * SW : (s - 4) * SW]

            # Output: stripe pairs (0,1),(2,3),(4,5) -> [128,1024] tiles
            # (one 256KB DMA for two stripes), stripes 6,7 -> [128,512]
            # singles for a tight drain.  DMA engine per tile chosen so the
            # backlog drains on whichever queue is free and the final
            # stripes land on the empty SP queue.
            pair_tiles = {}

            for s in range(NS):
                pss = [
                    pp.tile([128, SW], mybir.dt.float32, tag="ps", name=f"ps_{s}_{d}")
                    for d in range(DT)
                ]
                if s < NS - 1:
                    # k-outer: latest possible input deadlines
                    for k in range(KC):
                        for d in range(DT):
                            nc.tensor.matmul(
                                pss[d][:],
                                m_sb[k][:, d * 128 : (d + 1) * 128],
                                xslice(k, s),
                                start=(k == 0),
                                stop=(k == KC - 1),
                            )
                else:
                    # last stripe d-outer: banks retire staggered -> short tail
                    for d in range(DT):
                        for k in range(KC):
                            nc.tensor.matmul(
                                pss[d][:],
                                m_sb[k][:, d * 128 : (d + 1) * 128],
                                xslice(k, s),
                                start=(k == 0),
                                stop=(k == KC - 1),
                            )
                if s < 6:
                    pair, par = divmod(s, 2)
                    if par == 0:
                        pair_tiles[pair] = [
                            op.tile([128, 2 * SW], dt_out, tag=f"pt{pair}{d}", name=f"pt{pair}_{d}")
                            for d in range(DT)
                        ]
                    for d in range(DT):
                        nc.scalar.copy(
                            pair_tiles[pair][d][:, par * SW : (par + 1) * SW], pss[d][:]
                        )
                    if par == 1:
                        # pair 0 -> Pool; pair 1 -> SP; pair 2 -> SP/Pool split
                        for d in range(DT):
                            if pair == 0:
                                oeng = nc.gpsimd
                            elif pair == 1:
                                oeng = nc.sync
                            else:
                                oeng = nc.sync if d < 2 else nc.gpsimd
                            oeng.dma_start(
                                out=yt[d * 128 : (d + 1) * 128, pair * 2 * SW : (pair + 1) * 2 * SW],
                                in_=pair_tiles[pair][d][:],
                            )
                else:
                    for d in range(DT):
                        ot = op.tile([128, SW], dt_out, tag=f"ot{s}{d}", name=f"ot{s}_{d}")
                        nc.scalar.copy(ot[:], pss[d][:])
                        if s == 6:
                            oeng = nc.sync if d < 2 else nc.gpsimd
                        else:
                            oeng = nc.sync
                        oeng.dma_start(
                            out=yt[d * 128 : (d + 1) * 128, s * SW : (s + 1) * SW],
                            in_=ot[:],
                        )
    nc.compile()
    return nc


def _fold_m(W_v, s_p, W_p, beta_p, W_o):
    """Host-side constant folding of the tiny parameter tensors into M."""
    W_v = np.asarray(W_v, dtype=np.float64)
    s_p = np.asarray(s_p, dtype=np.float64)
    W_p = np.asarray(W_p, dtype=np.float64)
    beta_p = np.asarray(beta_p, dtype=np.float64)
    W_o = np.asarray(W_o, dtype=np.float64)
    phi = np.einsum("h,dhc,d->hc", s_p, W_p, beta_p)
    norma = np.linalg.norm(phi, axis=1)  # [h]
    M = np.einsum("dhc,h,hce->de", W_v, norma, W_o)  # [512, 512]
    return M.astype(np.float32)


_prog_cache = {}
_last_in_maps = None  # kept for test.py profiling reuse
_last_result = None


def _run(in_maps, token, **kwargs):
    if token not in _prog_cache:
        _prog_cache[token] = _build(token)
    return run_bass_kernel_spmd(_prog_cache[token], in_maps, list(range(N_CORES)), **kwargs)


def kernel(x, W_v, s_p, c_p, W_p, W_A, W_o, beta_p, beta_i_p, **_unused):
    global _last_in_maps, _last_result
    token = COMPUTE_DTYPE
    np_dt = _np_dtype(token)

    x = np.asarray(x, dtype=np.float32)
    M = _fold_m(W_v, s_p, W_p, beta_p, W_o)

    # fp16 path: scale M by an exact power of two so M entries and y values
    # sit in fp16 normal range; undo on the host after the run
    out_unscale = 1.0
    if token == "fp16":
        amax = float(np.abs(M).max())
        if amax > 0:
            e = int(np.floor(-np.log2(amax)))
            M = M * np.float32(2.0**e)
            out_unscale = 2.0**-e

    B, N, Dd = x.shape
    assert B * N == ROWS and Dd == D, (x.shape,)

    mmc = np.ascontiguousarray(M.reshape(KC, 128, D)).astype(np_dt)
    xf = x.reshape(ROWS, D)

    in_maps = []
    for c in range(N_CORES):
        sh = xf[c * RPC : (c + 1) * RPC]               # [4096, 512]
        xT = sh.T.astype(np_dt)                        # [512, 4096]
        xk = xT.reshape(KC, 128, NS, SW)               # [k, part, stripe, col]
        xs0 = np.ascontiguousarray(xk[:, :, 0:2].transpose(0, 2, 1, 3))
        xmid = np.ascontiguousarray(xk[:, :, 2:5].reshape(KC, 128, 3 * SW))
        xend = np.ascontiguousarray(xk[:, :, 5:8].reshape(KC, 128, 3 * SW))
        in_maps.append({"xs0": xs0, "xmid": xmid, "xend": xend, "mm": mmc})

    _last_in_maps = in_maps
    res = _run(in_maps, token)
    _last_result = res
    out = np.empty((ROWS, D), dtype=np.float32)
    for c in range(N_CORES):
        yc = res.results[c]["yt"].astype(np.float32)
        if out_unscale != 1.0:
            yc *= np.float32(out_unscale)
        out[c * RPC : (c + 1) * RPC] = yc.T
    return out.reshape(B, N, D)


if __name__ == "__main__":
    # smoke test with random data
    rng = np.random.default_rng(0)
    x = rng.standard_normal((8, 4096, 512)).astype(np.float32)
    W_v = rng.standard_normal((512, 8, 64)).astype(np.float32) * 0.01
    s_p = np.ones((8,), np.float32)
    c_p = np.ones((8,), np.float32)
    W_p = rng.standard_normal((512, 8, 64)).astype(np.float32) * 0.01
    W_A = rng.standard_normal((256, 64)).astype(np.float32)
    W_o = rng.standard_normal((8, 64, 512)).astype(np.float32) * 0.01
    beta_p = rng.standard_normal((512,)).astype(np.float32) * 1e-5
    beta_i_p = rng.standard_normal((4096, 512)).astype(np.float32) * 1e-5
    out = kernel(x, W_v=W_v, s_p=s_p, c_p=c_p, W_p=W_p, W_A=W_A, W_o=W_o,
                 beta_p=beta_p, beta_i_p=beta_i_p)
    M = _fold_m(W_v, s_p, W_p, beta_p, W_o)
    exp = (x.reshape(-1, 512).astype(np.float64) @ M.astype(np.float64)).reshape(8, 4096, 512)
    err = np.abs(out - exp).max() / (np.abs(exp).max() + 1e-30)
    print("smoke rel err:", err)
